# revision 64
# baseline (speedup 1.0000x reference)
"""Causal multi-head attention (RoPE) on 8 TRN2 NeuronCores.

Problem: x[2,2048,2048] -> qkv proj -> rope -> causal attention (16 heads,
head_dim 128) -> output proj + bias. Sharding: (batch, head-group) across the
8 cores - core c handles batch c//4 and heads 4*(c%4)..4*(c%4)+3. Each core
computes a partial output projection over its heads' channels; the host sums
the 4 partials per batch and adds b_o.

Fully SBUF-resident pipeline, no DRAM scratch roundtrips. The QKV
contraction runs in 2 passes of 8 c-tiles each (9 rotating x^T slots);
pass-0 partials evict via ACT copy; pass-1 q/k partials merge back into
PSUM with an identity matmul (keeps the DVE stream clear for the rope
chains) and evict via ACT, while v partials merge with a DVE add.
RoPE is applied in place (partition half-swap via SBUF->SBUF DMAs, sign
folded into sinT host-side), with chains emitted inside the attention jb
loop so the per-engine instruction streams interleave. q^T,k^T (all 4
heads) and batched v stay in SBUF through attention; ctx stays in SBUF
through the output projection, which runs fused per 512-token block.

All matmuls keep K (contraction) on partitions:
  - q,k produced transposed [d, tok]; v produced natural [tok, (h,d)]
  - scores computed transposed s^T[tk, tq] (lhsT=k^T tile, rhs=q^T block):
    softmax needs no transposes; exp on ACT; row-sums on DVE (lacc),
    partition-reduced and broadcast back via tiny ones-matmuls
  - AV: ctx^T[d, tq] = v.T @ p^T with PSUM accumulation over tk tiles
  - outproj: y[tok, o] accumulates the 4 heads' ctx^T.T @ W_o^T slices
Matmuls run in float32r (1 cycle/row at N>=256 vs 4 for fp32).
"""
import math

import numpy as np

import concourse.bacc as bacc
import concourse.mybir as mybir
import concourse.tile as tile
from concourse.bass_utils import run_bass_kernel_spmd

P = 128           # partitions / head_dim
T = 2048          # context length
C = 2048          # d_model
NKT = C // P      # 16 contraction tiles
NTT = T // P      # 16 token tiles
NB = T // 512     # 4 query blocks of 512
HPC = 4           # heads per core
NPASS = 2         # contraction passes
KPP = NKT // NPASS  # c-tiles per pass
NCORES = 8
SCALE = 1.0 / math.sqrt(P)
MASK_NEG = -1.0e30

F32 = mybir.dt.float32
F32R = mybir.dt.float32r
EXP = mybir.ActivationFunctionType.Exp
MULT = mybir.AluOpType.mult
ADD = mybir.AluOpType.add

_CACHE = {}


def _build(matmul_dt=F32R):
    nc = bacc.Bacc("TRN2", target_bir_lowering=False, debug=False,
                   num_devices=NCORES)
    dt = matmul_dt
    xT = nc.dram_tensor("xT", (C, T), dt, kind="ExternalInput").ap()
    wq = nc.dram_tensor("wq", (HPC, P, NKT, P), dt, kind="ExternalInput").ap()
    wk = nc.dram_tensor("wk", (HPC, P, NKT, P), dt, kind="ExternalInput").ap()
    wv = nc.dram_tensor("wv", (NKT, P, HPC * P), dt, kind="ExternalInput").ap()
    wo = nc.dram_tensor("wo", (HPC, P, C), dt, kind="ExternalInput").ap()
    cosT = nc.dram_tensor("cosT", (P, T), F32, kind="ExternalInput").ap()
    sinT = nc.dram_tensor("sinT", (P, T), F32, kind="ExternalInput").ap()
    tri = nc.dram_tensor("tri", (P, P), F32, kind="ExternalInput").ap()
    m3 = nc.dram_tensor("m3", (P, 2 * P), F32, kind="ExternalInput").ap()
    ones = nc.dram_tensor("ones", (P, P), dt, kind="ExternalInput").ap()
    eye = nc.dram_tensor("eye", (P, P), dt, kind="ExternalInput").ap()
    y = nc.dram_tensor("y", (T, C), F32, kind="ExternalOutput").ap()

    with tile.TileContext(nc) as tc:
        with (
            tc.tile_pool(name="gconst", bufs=1) as gpool,
            tc.tile_pool(name="qkbuf", bufs=1) as qkpool,
            tc.tile_pool(name="vbuf", bufs=1) as vpool,
        ):
            tri_sb = gpool.tile([P, P], F32, tag="tri")
            m3_sb = gpool.tile([P, 2 * P], F32, tag="m3")
            ones_sb = gpool.tile([P, P], dt, tag="ones")
            eye_sb = gpool.tile([P, P], dt, tag="eye")

            # persistent activations (SBUF-resident across phases)
            qk_sb = {}
            for h in range(HPC):
                for part in ("q", "k"):
                    for nb in range(NB):
                        t_ = qkpool.tile([P, 512], dt, tag=f"{part}{h}n{nb}",
                                         name=f"{part}{h}n{nb}_sb")
                        qk_sb[(part, h, nb)] = t_
            v_sb = [vpool.tile([P, HPC * P], dt, tag=f"vb{i}", name=f"v{i}_sb")
                    for i in range(NTT)]

            # ------------- Phase 1: QKV projection (4 passes) + rope --------
            with (
                tc.tile_pool(name="xp", bufs=1) as xpool,
                tc.tile_pool(name="wp", bufs=2) as wpool,
                tc.tile_pool(name="psv", bufs=1, space="PSUM") as psv,
                tc.tile_pool(name="ps1", bufs=2, space="PSUM") as ps1,
            ):
                xt_pref = {}
                for ps in range(NPASS):
                    if ps == 1:
                        nc.sync.dma_start(eye_sb[:], eye)
                        nc.sync.dma_start(tri_sb[:], tri)
                        nc.sync.dma_start(m3_sb[:], m3)
                        nc.sync.dma_start(ones_sb[:], ones)
                    kts = [ps * KPP + j for j in range(KPP)]
                    xt = {}
                    wvt = {}
                    w_tiles = {}

                    def load_w(h, part, wtens):
                        tiles = []
                        for half in range(2):
                            wt = wpool.tile([P, KPP // 2, P], dt, tag="w",
                                            bufs=4, name=f"w_{part}{h}_{half}")
                            nc.sync.dma_start(
                                wt[:], wtens[h][:, ps * KPP + half * (KPP // 2):
                                                ps * KPP + (half + 1) * (KPP // 2), :])
                            tiles.append(wt)
                        w_tiles[(part, h)] = tiles

                    for kt in kts:
                        if kt in xt_pref:
                            xt[kt] = xt_pref.pop(kt)
                        else:
                            x_ = xpool.tile([P, T], dt, tag=f"x{kt % 9}",
                                            bufs=1, name=f"x_{kt}")
                            nc.sync.dma_start(x_[:], xT[kt * P:(kt + 1) * P, :])
                            xt[kt] = x_
                        wv_ = wpool.tile([P, HPC * P], dt, tag=f"wv{kt % KPP}",
                                         bufs=1, name=f"wv_{kt}")
                        nc.sync.dma_start(wv_[:], wv[kt])
                        wvt[kt] = wv_
                        if kt == kts[1]:
                            load_w(0, "q", wq)
                        elif kt == kts[3]:
                            load_w(0, "k", wk)
                        elif ps == 0 and kt == kts[-1]:
                            # prefetch pass-1's first x tile into the spare slot
                            pk = KPP
                            px = xpool.tile([P, T], dt, tag=f"x{pk % 9}",
                                            bufs=1, name=f"x_{pk}")
                            nc.sync.dma_start(px[:], xT[pk * P:(pk + 1) * P, :])
                            xt_pref[pk] = px

                    # v: 4 tok-groups of 4 PSUM banks
                    for vg in range(4):
                        vaccs = []
                        for j, kt in enumerate(kts):
                            for i in range(4):
                                tt = vg * 4 + i
                                if j == 0:
                                    va = psv.tile([P, 512], F32, tag=f"v{i}",
                                                  bufs=1, name=f"va{ps}_{vg}_{i}")
                                    vaccs.append(va)
                                nc.tensor.matmul(
                                    vaccs[i][:],
                                    xt[kt][:, tt * P:(tt + 1) * P], wvt[kt][:],
                                    start=(j == 0), stop=(j == KPP - 1))
                        for i in range(4):
                            tt = vg * 4 + i
                            if ps == 0:
                                nc.scalar.copy(v_sb[tt][:], vaccs[i][:])
                            else:
                                nc.vector.tensor_tensor(
                                    v_sb[tt][:], v_sb[tt][:], vaccs[i][:],
                                    op=ADD)

                    # q,k: 8 (head, part) x 4 token-blocks
                    for h in range(HPC):
                        for part, wtens in (("q", wq), ("k", wk)):
                            if (part, h) not in w_tiles:
                                load_w(h, part, wtens)
                            wts = w_tiles[(part, h)]
                            for nb in range(NB):
                                dst = qk_sb[(part, h, nb)]
                                tsl = slice(nb * 512, (nb + 1) * 512)
                                acc = ps1.tile([P, 512], F32, tag="qk", bufs=4)
                                for j, kt in enumerate(kts):
                                    nc.tensor.matmul(
                                        acc[:], wts[j // (KPP // 2)][:, j % (KPP // 2), :],
                                        xt[kt][:, tsl],
                                        start=(j == 0),
                                        stop=(ps == 0 and j == KPP - 1))
                                if ps == 0:
                                    nc.scalar.copy(dst[:], acc[:])
                                else:
                                    nc.tensor.matmul(acc[:], eye_sb[:], dst[:],
                                                     start=False, stop=True)
                                    nc.scalar.copy(dst[:], acc[:])

            # ------------- Phase 2: attention fused with outproj ------------
            # jb outer / head inner; after each jb the output projection for
            # that token block runs, overlapping the next block's attention.
            with (
                tc.tile_pool(name="wop", bufs=1) as wopool,
                tc.tile_pool(name="ctxp", bufs=2) as ctxpool,
                tc.tile_pool(name="lp", bufs=2) as lpool,
                tc.tile_pool(name="pp", bufs=3) as ppool,
                tc.tile_pool(name="cxs", bufs=2) as cxspool,
                tc.tile_pool(name="yp", bufs=4) as ypool,
                tc.tile_pool(name="ps2s", bufs=4, space="PSUM") as ps2s,
                tc.tile_pool(name="ps2c", bufs=1, space="PSUM") as ps2c,
                tc.tile_pool(name="ps2l", bufs=1, space="PSUM") as ps2l,
                tc.tile_pool(name="ps3", bufs=2, space="PSUM") as ps3,
            ):
                # rope setup: rotate_half sign folded into sinT on host;
                # the half-swap is two SBUF->SBUF DMAs. Chains are emitted
                # inside the jb loop so per-engine streams interleave. Only
                # the nb=0 cos/sin chunks load before jb0's rope; wo and the
                # remaining chunks queue behind jb0's swaps.
                rope_cm1 = tc.tile_pool(name="rconst", bufs=1)
                rpool = rope_cm1.__enter__()
                rope_cm2 = tc.tile_pool(name="st", bufs=2)
                spool = rope_cm2.__enter__()
                cos_sb, sin_sb = [], []

                def load_cs(cnb):
                    csl = slice(cnb * 512, (cnb + 1) * 512)
                    c_ = rpool.tile([P, 512], F32, tag=f"cos{cnb}", name=f"cos{cnb}")
                    nc.sync.dma_start(c_[:], cosT[:, csl])
                    cos_sb.append(c_)
                    s_ = rpool.tile([P, 512], F32, tag=f"sin{cnb}", name=f"sin{cnb}")
                    nc.sync.dma_start(s_[:], sinT[:, csl])
                    sin_sb.append(s_)

                load_cs(0)
                half = P // 2
                wo_sb = []

                def rope_chunk(part, h, nb):
                    src = qk_sb[(part, h, nb)]
                    tmp = spool.tile([P, 512], dt, tag="rt", bufs=2, name="tmp")
                    nc.sync.dma_start(tmp[0:half, :], src[half:P, :])
                    nc.sync.dma_start(tmp[half:P, :], src[0:half, :])
                    t1 = spool.tile([P, 512], F32, tag="t1", bufs=2)
                    nc.gpsimd.tensor_tensor(t1[:], src[:], cos_sb[nb][:], op=MULT)
                    t2 = spool.tile([P, 512], F32, tag="t2", bufs=2)
                    nc.vector.tensor_tensor(t2[:], tmp[:], sin_sb[nb][:], op=MULT)
                    nc.vector.tensor_tensor(src[:], t1[:], t2[:], op=ADD)

                for jb in range(NB):
                    for h in range(HPC):
                        rope_chunk("k", h, jb)
                        rope_chunk("q", h, jb)
                    if jb == 0:
                        for cnb in range(1, NB):
                            load_cs(cnb)
                        for h in range(HPC):
                            w_sb = wopool.tile([P, C], dt, tag=f"wo{h}",
                                               name=f"wo{h}_sb")
                            nc.sync.dma_start(w_sb[:], wo[h])
                            wo_sb.append(w_sb)

                    nt = 4 * (jb + 1)
                    qsl = slice(jb * 512, (jb + 1) * 512)
                    ctx_tiles = {}
                    for h in range(HPC):
                        qT_sb = qk_sb[("q", h, jb)]
                        ctx_ps = ps2c.tile([P, 512], F32, tag="ctx", bufs=1)
                        lps = ps2l.tile([1, 512], F32, tag="l", bufs=1)
                        for i in range(nt):
                            r = i - 4 * jb
                            # causal narrowing: only tq >= tk contribute;
                            # r==3 keeps cols 256: with a memset for 256:384
                            c0 = 0 if r < 1 else (r * P if r <= 2 else 2 * P)
                            osl = slice(c0, 512)
                            kch = qk_sb[("k", h, i // 4)]
                            sps = ps2s.tile([P, 512], F32, tag="s", bufs=4)
                            nc.tensor.matmul(
                                sps[:, osl],
                                kch[:, (i % 4) * P:(i % 4 + 1) * P],
                                qT_sb[:, osl], start=True, stop=True)
                            pt = ppool.tile([P, 512], dt, tag="pt", bufs=6)
                            nc.scalar.activation(pt[:, osl], sps[:, osl], EXP,
                                                 scale=SCALE)
                            if 0 <= r <= 2:
                                dsl = slice(r * P, (r + 1) * P)
                                nc.gpsimd.tensor_tensor(
                                    pt[:, dsl], pt[:, dsl], tri_sb[:], op=MULT)
                            elif r == 3:
                                nc.gpsimd.tensor_tensor(
                                    pt[:, 2 * P:4 * P], pt[:, 2 * P:4 * P],
                                    m3_sb[:], op=MULT)
                            nc.tensor.matmul(
                                ctx_ps[:, osl],
                                v_sb[i][:, h * P:(h + 1) * P], pt[:, osl],
                                start=(i == 0), stop=(i == nt - 1))
                            nc.tensor.matmul(lps[:, osl], ones_sb[:, 0:1],
                                             pt[:, osl],
                                             start=(i == 0), stop=(i == nt - 1))
                        rinv = lpool.tile([1, 512], dt, tag="rinv", bufs=2)
                        with nc.allow_low_precision(reason="softmax 1/l fp32r"):
                            nc.vector.reciprocal(rinv[:], lps[:])
                        bps = ps3.tile([P, 512], F32, tag="y", bufs=2,
                                       name="bps")
                        nc.tensor.matmul(bps[:], ones_sb[0:1, :], rinv[:],
                                         start=True, stop=True)
                        cvt = cxspool.tile([P, 512], F32, tag="cvt")
                        nc.vector.tensor_copy(cvt[:], ctx_ps[:])
                        ctx_sb = ctxpool.tile([P, 512], dt, tag=f"cx{h}",
                                              bufs=2, name=f"ctx{h}_{jb}")
                        nc.vector.tensor_tensor(ctx_sb[:], cvt[:], bps[:], op=MULT)
                        ctx_tiles[h] = ctx_sb

                    # outproj for this token block
                    for sub in range(4):
                        tt = jb * 4 + sub
                        ssl = slice(sub * P, (sub + 1) * P)
                        for ob in range(NB):
                            yps = ps3.tile([P, 512], F32, tag="y", bufs=2)
                            for h in range(HPC):
                                nc.tensor.matmul(
                                    yps[:], ctx_tiles[h][:, ssl],
                                    wo_sb[h][:, ob * 512:(ob + 1) * 512],
                                    start=(h == 0), stop=(h == HPC - 1))
                            y_sb = ypool.tile([P, 512], F32, tag="ysb", bufs=4)
                            nc.vector.tensor_copy(y_sb[:], yps[:])
                            nc.sync.dma_start(
                                y[tt * P:(tt + 1) * P, ob * 512:(ob + 1) * 512],
                                y_sb[:])
                rope_cm2.__exit__(None, None, None)
                rope_cm1.__exit__(None, None, None)

    nc.compile()
    return nc


def _build_kernel(matmul_dt=F32R):
    key = str(matmul_dt)
    if key not in _CACHE:
        _CACHE[key] = _build(matmul_dt)
    return _CACHE[key]


def _host_constants():
    tri01 = (np.arange(P)[:, None] <= np.arange(P)[None, :]).astype(np.float32)
    m3 = np.concatenate([np.zeros((P, P), np.float32), tri01], axis=1)
    ones = np.ones((P, P), dtype=np.float32)
    eye = np.eye(P, dtype=np.float32)
    return tri01, m3, ones, eye


def prepare_in_maps(x, W_qkv, W_o, cos, sin):
    tri, m3, ones, eye = _host_constants()
    cosT = np.ascontiguousarray(cos.T)
    # rotate_half sign folded in: rows (head dims) 0..63 negated
    sgn = np.where(np.arange(P) < P // 2, -1.0, 1.0).astype(np.float32)
    sinT = np.ascontiguousarray(sin.T * sgn[:, None])

    in_maps = []
    for core in range(NCORES):
        b = core // 4
        hg0 = (core % 4) * HPC
        rows = slice(hg0 * P, (hg0 + HPC) * P)
        xTc = np.ascontiguousarray(x[b].T)
        wq_r = W_qkv[0 * C:1 * C][rows]        # [512, 2048]
        wk_r = W_qkv[1 * C:2 * C][rows]
        wv_r = W_qkv[2 * C:3 * C][rows]
        # (h, c_in_tile, kt, d) from W^T [2048(c), 512(h,d)]
        wq_t = np.ascontiguousarray(
            wq_r.T.reshape(NKT, P, HPC, P).transpose(2, 1, 0, 3))
        wk_t = np.ascontiguousarray(
            wk_r.T.reshape(NKT, P, HPC, P).transpose(2, 1, 0, 3))
        wv_t = np.ascontiguousarray(wv_r.T.reshape(NKT, P, HPC * P))
        wo_t = np.ascontiguousarray(W_o[:, rows].T.reshape(HPC, P, C))
        in_maps.append({
            "xT": xTc, "wq": wq_t, "wk": wk_t, "wv": wv_t, "wo": wo_t,
            "cosT": cosT, "sinT": sinT, "tri": tri, "m3": m3, "ones": ones, "eye": eye,
        })
    return in_maps


def gather(results, b_o):
    y = np.zeros((2, T, C), dtype=np.float32)
    for core in range(NCORES):
        y[core // 4] += results[core]["y"]
    y += np.asarray(b_o, dtype=np.float32)[None, None, :]
    return y


def kernel(x, W_qkv, W_o, b_o, cos, sin):
    x = np.asarray(x, dtype=np.float32)
    W_qkv = np.asarray(W_qkv, dtype=np.float32)
    W_o = np.asarray(W_o, dtype=np.float32)
    cos = np.asarray(cos, dtype=np.float32)
    sin = np.asarray(sin, dtype=np.float32)
    nc = _build_kernel()
    in_maps = prepare_in_maps(x, W_qkv, W_o, cos, sin)
    res = run_bass_kernel_spmd(nc, in_maps, core_ids=list(range(NCORES)))
    return gather(res.results, b_o)
